# revision 1
# baseline (speedup 1.0000x reference)
"""MultiHeadAttention + RoPE kernel for 8 Trainium2 NeuronCores.

Sharding: core c in 0..7 -> batch b = c//4, head-group hg = c%4 (4 heads
each).  Each core computes its 4 heads' attention for its batch and a
partial output projection y_part = out_heads @ wo[head rows]; the host
sums the 4 partials per batch and adds bo + bv@wo (the V bias is dropped
on device: out = PV/den + bv, so its y contribution is a constant row).

Per-core dataflow (all matmuls fp16 in / fp32 PSUM accumulation; the
exp softmax scale A = 2^7/ln2 is pre-folded into wq/bq on the host so
scores arrive as A*s_raw/8):
  - phase A: Q projected k-outer (4 open [128,1024] PSUM groups) so
    matmuls consume the streaming x k-chunks as they arrive; K groups
    software-pipelined one behind their rope evacs; PSUM evac via ACT
    Identity + per-partition bias AP; RoPE rotate-half is 4 partition-
    offset SBUF->SBUF DMA copies (signs pre-folded into the sin table),
    then 3 fp16 DVE ops (t1=q*cos, t2=rot*sin, add)
  - V natural [keys, depth] + an appended ones column -> V' [*, 65]
  - phase B per (qb, 512-half, h) unit: 16 score matmuls ST [128 keys,
    512 q] through a 6-slot PSUM ring; exp split 11/5 between ACT (true
    exp, scale=1/A) and DVE (fp16 Schraudolph: one tensor_scalar
    (s*8 + 15324) -> int16, bitcast fp16); pv matmuls emitted 2 slots
    late (software pipeline) so the PE FIFO never waits on exp; PV
    accumulates out'T [65, 512], row 64 = softmax denominator
  - normalize: den row copied to SBUF (reciprocal_approx_fast misreads
    PSUM), recip, gpsimd partition-broadcast, f32 mul into a [128,512]
    staging tile (partial-partition fp16 tensor ops fail walrus), one
    full-partition f32->fp16 copy into outT per mt pair
  - phase C per q-tile: y = outT^T @ wo (K=128 chunks over the 4 local
    heads), evac alternating ACT/DVE, deferred via a need-counted queue
    so it drains under the next block's attention without deadlocking
    the DVE FIFO; y returned fp16, upconverted + reduced on the host
"""

import collections
import math

import numpy as np

import concourse.bacc as bacc
import concourse.mybir as mybir
from concourse.tile import TileContext

try:  # persistent XLA compile cache: repeat processes skip the ~4min compile
    import jax as _jax
    _jax.config.update("jax_compilation_cache_dir", "/tmp/jax_comp_cache")
    _jax.config.update("jax_persistent_cache_min_compile_time_secs", 1.0)
except Exception:
    pass

B, S, DM, H, DH = 2, 2048, 1024, 16, 64
NCORES = 8
HL = 4                # heads per core
DHL = HL * DH         # 256
KCH = DM // 128       # 8 k-chunks of the model-dim contraction
SKT = S // 128        # 16 key tiles
QT_TILES = DHL // 128  # 2 m-tiles for the Q/K projections
QB = 1024              # q block (half the rope-evac granularity)
NQB = S // QB

AEXP = 128.0 / math.log(2.0)      # Schraudolph exponent scale 2^7/ln2
QSCALE = AEXP / 8.0               # folded into wq/bq: scores arrive as A*s/8
SCHR_C = 36.0                     # fp16 Schraudolph bias correction (tuned)
SCHR_B = 15.0 * 1024.0 - SCHR_C   # fp16 exponent bias field
SCHR_MUL = 8.0                    # rescale s_pre to 2^10/ln2 units
ACT_SCALE = 1.0 / AEXP

F32 = mybir.dt.float32
BF16 = mybir.dt.float16  # "BF16" label kept; fp16 halves the rounding error
I16 = mybir.dt.int16
EXP = mybir.ActivationFunctionType.Exp
COPY = mybir.ActivationFunctionType.Copy
IDENT = mybir.ActivationFunctionType.Identity
ADD = mybir.AluOpType.add
MULT = mybir.AluOpType.mult

# per-16 exp engine pattern: 11 ACT ("a"), 5 DVE ("d")
EXP_PATTERN = "aadaadaadaadaada"

_CACHE = {}


def _build_nc(debug=False):
    nc = bacc.Bacc()
    xT = nc.dram_tensor("xT", [DM, S], BF16, kind="ExternalInput")
    wq = nc.dram_tensor("wq", [DM, DHL], BF16, kind="ExternalInput")
    wk = nc.dram_tensor("wk", [DM, DHL], BF16, kind="ExternalInput")
    wv = nc.dram_tensor("wv", [DM, DHL], BF16, kind="ExternalInput")
    wo = nc.dram_tensor("wo", [128, QT_TILES, DM], BF16, kind="ExternalInput")
    bq = nc.dram_tensor("bq", [128, QT_TILES], F32, kind="ExternalInput")
    bk = nc.dram_tensor("bk", [128, QT_TILES], F32, kind="ExternalInput")
    cosT = nc.dram_tensor("cosT", [128, S], BF16, kind="ExternalInput")
    sinT = nc.dram_tensor("sinT", [128, S], BF16, kind="ExternalInput")
    y = nc.dram_tensor("y", [S, DM], BF16, kind="ExternalOutput")
    if debug:
        qrope_d = nc.dram_tensor("qrope_d", [128, QT_TILES, S], BF16,
                                 kind="ExternalOutput")
        krope_d = nc.dram_tensor("krope_d", [128, QT_TILES, S], BF16,
                                 kind="ExternalOutput")
        v_d = nc.dram_tensor("v_d", [128, SKT, HL, DH + 1], BF16,
                             kind="ExternalOutput")
        outT_d = nc.dram_tensor("outT_d", [128, QT_TILES, S], BF16,
                                kind="ExternalOutput")

    with TileContext(nc) as tc:
        with tc.tile_pool(name="p0", bufs=1) as p0:
            qrope_r = p0.tile([128, QT_TILES, S], BF16)
            krope_r = p0.tile([128, QT_TILES, S], BF16)
            v_r = p0.tile([128, SKT, HL, DH + 1], BF16)
            outT_r = p0.tile([128, QT_TILES, S], BF16)
            wo_r = p0.tile([128, QT_TILES, DM], BF16)
            ones_col = p0.tile([128, 1], BF16)
            nc.vector.memset(ones_col[:], 1.0)

            # ================= PHASE A =================
            # (single flat pool scope: phase B shares the PSUM tags so no
            # pool-release barrier separates the phases)
            with (
                tc.tile_pool(name="pa", bufs=1) as pa,
                tc.tile_pool(name="pa_t", bufs=4) as pa_t,
                tc.tile_pool(name="ps_b", bufs=1, space="PSUM") as ps_b,
                tc.tile_pool(name="pb_exp", bufs=8) as pb_exp,
                tc.tile_pool(name="pb_n", bufs=3) as pb_n,
                tc.tile_pool(name="pc_y", bufs=6) as pc_y,
            ):
                # PE pstate warm-up: the cost model halves the PE clock
                # until ~3us after the first matmul; burn the DMA-latency
                # head with dummies so the first real ones run full speed
                wsrc = pa.tile([128, 512], BF16, tag="wsrc")
                nc.vector.memset(wsrc[:], 1.0)
                wps = ps_b.tile([128, 512], F32, tag="st", bufs=6,
                                name="wps")
                for _ in range(7):
                    nc.tensor.matmul(wps[:], wsrc[:, 0:128], wsrc[:],
                                     start=True, stop=True)

                # DMA order: wq then x chunks first so PE starts ASAP
                # (wq and the first x chunk split so the first matmul's
                # serialized DMA-transfer chain is as short as possible)
                wq_r = pa.tile([128, KCH, DHL], BF16, tag="wqr")
                wq_re = wq.rearrange("(k p) n -> p k n", p=128)
                nc.sync.dma_start(wq_r[:, 0:2, :], wq_re[:, 0:2, :])
                xT_r = pa.tile([128, KCH, S], BF16)
                nc.sync.dma_start(xT_r[:, 0, 0:QB], xT[0:128, 0:QB])
                nc.sync.dma_start(xT_r[:, 0, QB:S], xT[0:128, QB:S])
                nc.sync.dma_start(xT_r[:, 1, 0:QB], xT[128:256, 0:QB])
                nc.sync.dma_start(xT_r[:, 1, QB:S], xT[128:256, QB:S])
                bq_sb = pa.tile([128, QT_TILES], F32, tag="bq")
                bk_sb = pa.tile([128, QT_TILES], F32, tag="bk")
                nc.sync.dma_start(bq_sb[:], bq[:, :])
                nc.sync.dma_start(bk_sb[:], bk[:, :])
                nc.sync.dma_start(wq_r[:, 2:3, :], wq_re[:, 2:3, :])
                nc.sync.dma_start(xT_r[:, 2, 0:QB], xT[256:384, 0:QB])
                nc.sync.dma_start(xT_r[:, 2, QB:S], xT[256:384, QB:S])
                nc.sync.dma_start(wq_r[:, 3:KCH, :], wq_re[:, 3:KCH, :])
                nc.sync.dma_start(xT_r[:, 3, 0:QB], xT[384:512, 0:QB])
                nc.sync.dma_start(xT_r[:, 3, QB:S], xT[384:512, QB:S])
                for k in range(4, KCH):
                    nc.sync.dma_start(xT_r[:, k, :], xT[k * 128:(k + 1) * 128, :])
                wk_r = pa.tile([128, KCH, DHL], BF16, tag="wkr")
                nc.sync.dma_start(
                    wk_r[:], wk.rearrange("(k p) n -> p k n", p=128))
                cos_sb = pa.tile([128, S], BF16)
                sin_sb = pa.tile([128, S], BF16)
                nc.sync.dma_start(cos_sb[:], cosT[:, :])
                nc.sync.dma_start(sin_sb[:], sinT[:, :])
                wv_r = pa.tile([128, KCH, DHL], BF16, tag="wvr")
                nc.sync.dma_start(
                    wv_r[:], wv.rearrange("(k p) n -> p k n", p=128))
                nc.sync.dma_start(wo_r[:], wo[:, :, :])

                # preload the exp ACT table while ACT is idle
                warm = pa.tile([1, 128], F32, tag="warm")
                nc.vector.memset(warm[:], 0.0)
                warm2 = pa.tile([1, 128], F32, tag="warm2")
                nc.scalar.activation(warm2[:], warm[:], EXP, scale=ACT_SCALE)

                # Q projection k-outer: 4 open PSUM groups [2mt x 2qb],
                # consuming x chunks as they stream in
                groups = [(mt, qb) for mt in range(QT_TILES)
                          for qb in range(NQB)]
                ab_tags = [("st", 6)] * 6 + [("pv", 2)] * 2

                def proj_halves(name):
                    t = {}
                    for i, (mt, qb) in enumerate(groups):
                        for nq in range(2):
                            tag, bufs = ab_tags[i * 2 + nq]
                            t[(mt, qb, nq)] = ps_b.tile(
                                [128, 512], F32, tag=tag, bufs=bufs,
                                name=f"{name}{mt}{qb}{nq}")
                    return t

                qps = proj_halves("qp")
                for k in range(KCH):
                    korder = (sorted(groups, key=lambda g: g[1])
                              if k == 0 else groups)
                    for mt, qb in korder:
                        for nq in range(QB // 512):
                            q0 = qb * QB + nq * 512
                            nc.tensor.matmul(
                                qps[(mt, qb, nq)][:, :],
                                wq_r[:, k, mt * 128:(mt + 1) * 128],
                                xT_r[:, k, q0:q0 + 512],
                                start=(k == 0), stop=(k == KCH - 1))

                def rope_evac(pair, mt, qb, b_sb, dest):
                    """PSUM halves 2x[128, 512] -> dest rope slice (fp16).

                    rotate-half is 4 partition-offset SBUF->SBUF DMA copies;
                    the rot signs live in the (pre-negated) sin table rows."""
                    q0 = qb * QB
                    qb_r = pa_t.tile([128, QB], BF16, tag="qbr")
                    for nq in range(2):
                        nc.scalar.activation(qb_r[:, nq * 512:(nq + 1) * 512],
                                             pair[nq][:], IDENT,
                                             bias=b_sb[:, mt:mt + 1])
                    qrot = pa_t.tile([128, QB], BF16, tag="qrot")
                    hh = DH // 2
                    for blk in range(4):
                        p0_ = blk * hh
                        src_ = p0_ + hh if blk % 2 == 0 else p0_ - hh
                        nc.sync.dma_start(qrot[p0_:p0_ + hh, :],
                                          qb_r[src_:src_ + hh, :])
                    t1 = pa_t.tile([128, QB], BF16, tag="t1")
                    nc.vector.tensor_mul(t1[:], qb_r[:],
                                         cos_sb[:, q0:q0 + QB])
                    t2 = pa_t.tile([128, QB], BF16, tag="t2")
                    nc.vector.tensor_mul(t2[:], qrot[:],
                                         sin_sb[:, q0:q0 + QB])
                    nc.vector.tensor_add(dest[:, mt, q0:q0 + QB],
                                         t1[:], t2[:])

                # Q rope-evacs for all groups, then K projections software-
                # pipelined one group ahead of their rope-evacs so PE always
                # has matmul work while ACT/DVE evac chains run
                for mt, qb in groups:
                    rope_evac((qps[(mt, qb, 0)], qps[(mt, qb, 1)]),
                              mt, qb, bq_sb, qrope_r)
                kps = {}
                for i, (mt, qb) in enumerate(groups):
                    for nq in range(2):
                        tag, bufs = ab_tags[i * 2 + nq]
                        kps[(mt, qb, nq)] = ps_b.tile(
                            [128, 512], F32, tag=tag, bufs=bufs,
                            name=f"kp{mt}{qb}{nq}")
                    for k in range(KCH):
                        for nq in range(QB // 512):
                            q0 = qb * QB + nq * 512
                            nc.tensor.matmul(
                                kps[(mt, qb, nq)][:, :],
                                wk_r[:, k, mt * 128:(mt + 1) * 128],
                                xT_r[:, k, q0:q0 + 512],
                                start=(k == 0), stop=(k == KCH - 1))
                    if i >= 1:
                        pmt, pqb = groups[i - 1]
                        rope_evac((kps[(pmt, pqb, 0)], kps[(pmt, pqb, 1)]),
                                  pmt, pqb, bk_sb, krope_r)
                g = groups[-1]
                rope_evac((kps[(g[0], g[1], 0)], kps[(g[0], g[1], 1)]),
                          g[0], g[1], bk_sb, krope_r)

                # V projection (natural layout), bias added during DVE evac
                nc.vector.tensor_copy(
                    v_r[:, :, :, DH:DH + 1],
                    ones_col[:, None, None, :].broadcast_to([128, SKT, HL, 1]))
                for sk in range(SKT):
                    vps = ps_b.tile([128, DHL], F32, tag="st", bufs=6,
                                    name="vps")
                    for k in range(KCH):
                        nc.tensor.matmul(
                            vps[:], xT_r[:, k, sk * 128:(sk + 1) * 128],
                            wv_r[:, k, :],
                            start=(k == 0), stop=(k == KCH - 1))
                    nc.scalar.activation(
                        v_r[:, sk, :, 0:DH],
                        vps[:].rearrange("p (h d) -> p h d", h=HL), COPY)

                # ================= PHASE B + C =================
                pend = collections.deque()   # delayed pv matmuls
                held = []                    # phase-C work held for the tail
                o32_cur = [None]             # f32 outT staging tile
                cq = collections.deque()     # deferred phase-C work

                norms = [0]                  # norm muls emitted so far

                def drain(pend_keep, c_budget):
                    while len(pend) > pend_keep:
                        pend.popleft()()
                    # C ops may only be emitted once every norm mul their
                    # outT read needs is already emitted, else the DVE FIFO
                    # deadlocks (C's evac would precede the norm mul)
                    while c_budget > 0 and cq and cq[0][0] <= norms[0]:
                        cq.popleft()[1]()
                        c_budget -= 1

                def emit_phase_c(qb, hf, need, dve_dma=False, sub=None):
                    """y q-tiles of one 512-half, deferred per q-tile."""
                    base = (qb * QB + hf * 512) // 128
                    qts = (range(base, base + 4) if sub is None else
                           range(base + sub * 2, base + sub * 2 + 2))
                    hold_n = 4 if (qb == NQB - 1 and hf == 0) else 0
                    for qt in qts:
                        def do_qt(qt=qt):
                            y_sb = pc_y.tile([128, DM], BF16, tag="ysb")
                            for c2 in range(2):
                                y_ps = ps_b.tile([128, 512], F32, tag="st",
                                                 bufs=6)
                                for kc in range(QT_TILES):
                                    nc.tensor.matmul(
                                        y_ps[:],
                                        outT_r[:, kc, qt * 128:(qt + 1) * 128],
                                        wo_r[:, kc, c2 * 512:(c2 + 1) * 512],
                                        start=(kc == 0),
                                        stop=(kc == QT_TILES - 1))
                                half = y_sb[:, c2 * 512:(c2 + 1) * 512]
                                if c2 == 0:
                                    nc.scalar.activation(half, y_ps[:], COPY)
                                else:
                                    nc.vector.tensor_copy(half, y_ps[:])
                            q_eng = nc.scalar if dve_dma else nc.sync
                            q_eng.dma_start(
                                y[qt * 128:(qt + 1) * 128, :], y_sb[:])
                        if qt - base >= 4 - hold_n:
                            held.append(do_qt)  # real work for the tail gap
                        else:
                            cq.append((need, do_qt))

                def emit_unit(h, q0, w, act_tail=False):
                    mt = h // 2
                    half = (h % 2) * DH
                    qt_h = qrope_r[half:half + DH, mt, :]
                    kt_h = krope_r[half:half + DH, mt, :]
                    pv_ps = ps_b.tile([DH + 1, w], F32, tag="pv", bufs=2,
                                      name="pv_ps")
                    for sk in range(SKT):
                        st = ps_b.tile([128, w], F32, tag="st", bufs=6,
                                       name="st")
                        nc.tensor.matmul(
                            st[:], kt_h[:, sk * 128:(sk + 1) * 128],
                            qt_h[:, q0:q0 + w], start=True, stop=True)
                        expst = pb_exp.tile([128, w], BF16, tag="expst",
                                            name="expst")
                        eng = EXP_PATTERN[(sidx[0] * 11) % 16]
                        sidx[0] += 1
                        if act_tail and sk >= 12:
                            # keep DVE free for the tail norm chain
                            eng = "a"
                        if eng == "a":
                            nc.scalar.activation(expst[:], st[:], EXP,
                                                 scale=ACT_SCALE)
                        else:
                            nc.vector.tensor_scalar(
                                out=expst[:].bitcast(I16), in0=st[:],
                                scalar1=SCHR_MUL, scalar2=SCHR_B,
                                op0=MULT, op1=ADD)

                        def do_pv(sk=sk, expst=expst, pv_ps=pv_ps, h=h):
                            nc.tensor.matmul(
                                pv_ps[:], v_r[:, sk, h, :], expst[:],
                                start=(sk == 0), stop=(sk == SKT - 1))
                        pend.append(do_pv)
                        # C matmuls enter the PE FIFO when drained; give the
                        # outT staging copy a head start so they never block
                        drain(2, 2 if sk >= 6 else 0)

                    # norm chain split into separate pend entries so its
                    # DVE ops spread across score slots instead of bursting
                    # ahead of queued exps
                    state = {}

                    def norm_recip(pv_ps=pv_ps, state=state, w=w):
                        # reciprocal_approx_fast misreads PSUM operands on
                        # HW: stage the denominator row through SBUF first
                        dent = pb_n.tile([1, w], F32, tag="dent")
                        nc.vector.tensor_copy(dent[:], pv_ps[DH:DH + 1, :])
                        recd = pb_n.tile([1, w], F32, tag="recd")
                        nc.vector.reciprocal_approx_fast(
                            out=recd[:], in_=dent[:])
                        state["recd"] = recd

                    def norm_bcast(state=state, w=w):
                        rec_b = pb_n.tile([DH, w], F32, tag="recb")
                        nc.gpsimd.partition_broadcast(
                            rec_b[:], state["recd"][0:1, :])
                        state["recb"] = rec_b

                    def norm_mul(pv_ps=pv_ps, half=half, mt=mt, q0=q0, w=w,
                                 state=state):
                        # partial-partition fp16 tensor ops fail walrus
                        # codegen: normalize in f32 into a 128-partition
                        # staging tile, convert to outT with one
                        # full-partition copy once both mt heads are done
                        if half == 0:
                            o32_cur[0] = pb_n.tile([128, w], F32, tag="o32",
                                                   bufs=3, name="o32")
                        nc.vector.tensor_mul(o32_cur[0][half:half + DH, :],
                                             pv_ps[0:DH, :], state["recb"])
                        if half == DH:
                            nc.vector.tensor_copy(outT_r[:, mt, q0:q0 + w],
                                                  o32_cur[0][:])
                        norms[0] += 1
                    pend.append(norm_recip)
                    pend.append(norm_bcast)
                    pend.append(norm_mul)

                sidx = [0]
                for qb in range(NQB):
                    for hf in range(2):
                        last_blk = qb == NQB - 1 and hf == 1
                        q0 = qb * QB + hf * 512
                        for h in range(HL):
                            emit_unit(h, q0, 512,
                                      act_tail=(last_blk and h == HL - 1))
                        emit_phase_c(qb, hf, need=(qb * 2 + hf + 1) * HL)
                drain(0, 0)
                # fill the final norm-chain wait with the held-back phase-C
                # tiles (their norms are long done), topped up with dummy
                # matmuls to keep the PE pstate warm at full clock
                for fn in held:
                    fn()
                while cq:
                    drain(0, 1)
                if debug:
                    nc.sync.dma_start(qrope_d[:, :, :], qrope_r[:])
                    nc.sync.dma_start(krope_d[:, :, :], krope_r[:])
                    nc.sync.dma_start(v_d[:, :, :, :], v_r[:])
                    nc.sync.dma_start(outT_d[:, :, :], outT_r[:])

    nc.finalize()
    return nc


def _rope_tables():
    """cos/sin tables [128, S]; sin rows d with d%64 < 32 are NEGATED so the
    rotate-half signs ride in the table (the device rot is 4 unsigned
    partition-block DMA copies)."""
    inv_freq = 1.0 / (10000.0 ** (np.arange(0, DH, 2, dtype=np.float32) / DH))
    ang = np.arange(S, dtype=np.float32)[:, None] * inv_freq[None, :]
    sin = np.concatenate([np.sin(ang), np.sin(ang)], axis=-1)  # [S, DH]
    cos = np.concatenate([np.cos(ang), np.cos(ang)], axis=-1)
    sinT = np.ascontiguousarray(np.vstack([sin.T, sin.T]), dtype=np.float32)
    cosT = np.ascontiguousarray(np.vstack([cos.T, cos.T]), dtype=np.float32)
    signs = np.where((np.arange(128) % DH) < (DH // 2), -1.0, 1.0)
    sinT = sinT * signs[:, None].astype(np.float32)
    return sinT, cosT  # [128, S]


def _make_runner(nc):
    """Build a cached jitted SPMD executor (mirrors the multi-core tail of
    concourse.bass2jax.run_bass_via_pjrt so repeat calls skip recompiles)."""
    import jax
    import numpy as _np
    from jax.sharding import Mesh, PartitionSpec
    from jax.experimental.shard_map import shard_map
    from concourse import bass2jax, mybir as _mybir

    bass2jax.install_neuronx_cc_hook()

    partition_name = (
        nc.partition_id_tensor.name if nc.partition_id_tensor else None)
    in_names, out_names, out_avals, zero_shapes = [], [], [], []
    for alloc in nc.m.functions[0].allocations:
        if not isinstance(alloc, _mybir.MemoryLocationSet):
            continue
        name = alloc.memorylocations[0].name
        if alloc.kind == "ExternalInput":
            if name != partition_name:
                in_names.append(name)
        elif alloc.kind == "ExternalOutput":
            out_names.append(name)
            shape = tuple(alloc.tensor_shape)
            dtype = _mybir.dt.np(alloc.dtype)
            out_avals.append(jax.core.ShapedArray(shape, dtype))
            zero_shapes.append((shape, dtype))
    n_params = len(in_names)
    all_names = in_names + out_names
    if partition_name is not None:
        all_names = all_names + [partition_name]

    def _body(*args):
        operands = list(args)
        if partition_name is not None:
            operands.append(bass2jax.partition_id_tensor())
        outs = bass2jax._bass_exec_p.bind(
            *operands,
            out_avals=tuple(out_avals),
            in_names=tuple(all_names),
            out_names=tuple(out_names),
            lowering_input_output_aliases=(),
            sim_require_finite=True,
            sim_require_nnan=True,
            nc=nc,
        )
        return tuple(outs)

    devices = jax.devices()[:NCORES]
    mesh = Mesh(_np.asarray(devices), ("core",))
    n_outs = len(out_names)
    sharded = jax.jit(
        shard_map(
            _body, mesh=mesh,
            in_specs=(PartitionSpec("core"),) * (n_params + n_outs),
            out_specs=(PartitionSpec("core"),) * n_outs,
            check_rep=False,
        ),
        donate_argnums=tuple(range(n_params, n_params + n_outs)),
        keep_unused=True,
    )

    def run(in_maps):
        concat_in = [
            _np.concatenate([_np.asarray(m[name]) for m in in_maps], axis=0)
            for name in in_names
        ]
        concat_zeros = [
            _np.zeros((NCORES * s[0], *s[1:]), dt) for (s, dt) in zero_shapes
        ]
        out_arrs = sharded(*concat_in, *concat_zeros)
        return [
            {
                name: _np.asarray(out_arrs[i]).reshape(
                    NCORES, *out_avals[i].shape)[c]
                for i, name in enumerate(out_names)
            }
            for c in range(NCORES)
        ]

    return run


def _get_runner():
    if "runner" not in _CACHE:
        nc = _build_nc()
        _CACHE["nc"] = nc
        _CACHE["runner"] = _make_runner(nc)
    return _CACHE["runner"]


def _bf16(a):
    return np.asarray(a, dtype=np.float32).astype(np.float16)


def make_in_maps(x, wq, bq, wk, bk, wv, bv, wo, bo):
    """Build the 8 per-core input dicts from full inputs."""
    x = np.asarray(x, dtype=np.float32)
    if "tables" not in _CACHE:
        _CACHE["tables"] = _rope_tables()
    sinT, cosT = _CACHE["tables"]
    in_maps = []
    for c in range(NCORES):
        b, hg = divmod(c, HL)
        sl = slice(hg * DHL, (hg + 1) * DHL)
        in_maps.append({
            "xT": _bf16(x[b].T),
            "wq": _bf16(np.asarray(wq, np.float32)[:, sl] * QSCALE),
            "wk": _bf16(np.asarray(wk, np.float32)[:, sl]),
            "wv": _bf16(np.asarray(wv, np.float32)[:, sl]),
            "wo": _bf16(
                np.asarray(wo, np.float32)[sl, :].reshape(QT_TILES, 128, DM)
                .transpose(1, 0, 2)),
            "bq": np.ascontiguousarray(
                (np.asarray(bq, np.float32)[sl] * QSCALE)
                .reshape(QT_TILES, 128).T),
            "bk": np.ascontiguousarray(
                np.asarray(bk, np.float32)[sl].reshape(QT_TILES, 128).T),
            "cosT": _bf16(cosT),
            "sinT": _bf16(sinT),
        })
    return in_maps


def kernel(x, wq, bq, wk, bk, wv, bv, wo, bo):
    runner = _get_runner()
    in_maps = make_in_maps(x, wq, bq, wk, bk, wv, bv, wo, bo)
    results = runner(in_maps)
    bo = np.asarray(bo, dtype=np.float32)
    # the device computes attention with V's bias omitted; its contribution
    # to y is the constant row bv @ wo, folded into the host-side bias add
    const_row = (np.asarray(bv, np.float32) @ np.asarray(wo, np.float32)
                 + bo).astype(np.float32)
    out = np.empty((B, S, DM), dtype=np.float32)
    for b in range(B):
        acc = results[b * HL + 0]["y"].astype(np.float32, copy=True)
        for hg in range(1, HL):
            acc += results[b * HL + hg]["y"]
        out[b] = acc + const_row[None, :]
    return out



# revision 7
# speedup vs baseline: 1.0651x; 1.0651x over previous
"""MultiHeadAttention + RoPE kernel for 8 Trainium2 NeuronCores.

Sharding: core c in 0..7 -> batch b = c//4, head-group hg = c%4 (4 heads
each).  Each core computes its 4 heads' attention for its batch and a
partial output projection y_part = out_heads @ wo[head rows]; the host
sums the 4 partials per batch and adds bo + bv@wo (the V bias is dropped
on device: out = PV/den + bv, so its y contribution is a constant row).

Per-core dataflow (matmuls fp16 in / fp32 PSUM accumulation; the
exp scale A = 2^7/ln2 is pre-folded into wq/bq on the host):
  - phase A: Q projected k-outer in two waves (wave1 = qb0 groups
    consume the streaming x chunks; wave2 runs dense), K groups
    software-pipelined one behind their rope evacs; PSUM evac via ACT
    Identity + per-partition bias AP; RoPE rotate-half is 4 partition-
    offset SBUF->SBUF DMA copies (signs pre-folded into the sin table),
    then 3 fp16 DVE ops; V natural [keys, depth] + ones column -> V'
  - phase B per (chunk, head) unit: 16 score matmuls ST [128 keys,
    512 q]; exp split ACT (true exp) / DVE / Pool (fp16 Schraudolph:
    tensor_scalar -> int16, bitcast fp16) into an E tile [128,16,512];
    PV is FLIPPED: stationary = E tile [128k, 128q] (weight loads are
    free), moving = V' [128k, 65] -> psum [128 q, 65] accumulated over
    keys; 65 cycles per matmul instead of 512.  Row 64 = softmax
    denominator -> per-partition reciprocal; one DVE mul normalizes
    into fp16 staging [128 q, qt, h2, 64]; PE transposes [128,128]
    (via identity) rebuild outT [2h*64d, q] exactly as the old layout
  - phase C per q-tile: y = outT^T @ wo (K=128 chunks over the 4 local
    heads), evac via Pool, deferred into the next chunk's score stream;
    y returned fp16, upconverted + reduced on the host
"""

import math

import numpy as np

import concourse.bacc as bacc
import concourse.mybir as mybir
from concourse.tile import TileContext

try:  # persistent XLA compile cache: repeat processes skip the ~4min compile
    import jax as _jax
    _jax.config.update("jax_compilation_cache_dir", "/tmp/jax_comp_cache")
    _jax.config.update("jax_persistent_cache_min_compile_time_secs", 1.0)
except Exception:
    pass

B, S, DM, H, DH = 2, 2048, 1024, 16, 64
NCORES = 8
HL = 4                # heads per core
DHL = HL * DH         # 256
KCH = DM // 128       # 8 k-chunks of the model-dim contraction
SKT = S // 128        # 16 key tiles
QT_TILES = DHL // 128  # 2 m-tiles for the Q/K projections
QB = 1024              # q block (rope-evac granularity)
NQB = S // QB
W = 512                # phase-B q chunk
NCHUNK = S // W        # 4 chunks

AEXP = 128.0 / math.log(2.0)      # exp scale folded into wq/bq
QSCALE = AEXP / 8.0
SCHR_C = 36.0                     # fp16 Schraudolph bias correction
SCHR_B = 15.0 * 1024.0 - SCHR_C
SCHR_MUL = 8.0
ACT_SCALE = 1.0 / AEXP

F32 = mybir.dt.float32
BF16 = mybir.dt.float16  # fp16: halves the rounding error vs bf16
I16 = mybir.dt.int16
EXP = mybir.ActivationFunctionType.Exp
COPY = mybir.ActivationFunctionType.Copy
IDENT = mybir.ActivationFunctionType.Identity
ADD = mybir.AluOpType.add
MULT = mybir.AluOpType.mult

# per-16 exp engine pattern: ACT (true exp) / DVE (Schraudolph).
# Pool (gpsimd) cannot read PSUM on TRN2, so it gets no exp tiles.
EXP_PAT = "adaadadadaadadad"

# (chunk, mt) head-pairs that use the original PV orientation
# (stationary V', out [65, 512]); the rest use the flipped PV
# (stationary E-tile, out [128, 65] -> PE transpose).  The flip trades
# PE-engine cycles (65 vs 512 per key tile) for PE-sequencer
# instructions (4 vs 1); the mix balances the two resources.
ORIG_PAIRS = ((0, 0), (1, 1), (2, 0))

_CACHE = {}


def _build_nc(debug=False):
    nc = bacc.Bacc()
    xT = nc.dram_tensor("xT", [DM, S], BF16, kind="ExternalInput")
    wq = nc.dram_tensor("wq", [DM, DHL], BF16, kind="ExternalInput")
    wk = nc.dram_tensor("wk", [DM, DHL], BF16, kind="ExternalInput")
    wv = nc.dram_tensor("wv", [DM, DHL], BF16, kind="ExternalInput")
    wo = nc.dram_tensor("wo", [128, QT_TILES, DM], BF16, kind="ExternalInput")
    bq = nc.dram_tensor("bq", [128, QT_TILES], F32, kind="ExternalInput")
    bk = nc.dram_tensor("bk", [128, QT_TILES], F32, kind="ExternalInput")
    cosT = nc.dram_tensor("cosT", [128, S], BF16, kind="ExternalInput")
    sinT = nc.dram_tensor("sinT", [128, S], BF16, kind="ExternalInput")
    ident = nc.dram_tensor("ident", [128, 128], BF16, kind="ExternalInput")
    y = nc.dram_tensor("y", [S, DM], BF16, kind="ExternalOutput")
    if debug:
        qrope_d = nc.dram_tensor("qrope_d", [128, QT_TILES, S], BF16,
                                 kind="ExternalOutput")
        krope_d = nc.dram_tensor("krope_d", [128, QT_TILES, S], BF16,
                                 kind="ExternalOutput")
        v_d = nc.dram_tensor("v_d", [128, SKT, HL, DH + 1], BF16,
                             kind="ExternalOutput")
        outT_d = nc.dram_tensor("outT_d", [128, QT_TILES, S], BF16,
                                kind="ExternalOutput")

    with TileContext(nc) as tc:
        with tc.tile_pool(name="p0", bufs=1) as p0:
            qrope_r = p0.tile([128, QT_TILES, S], BF16)
            krope_r = p0.tile([128, QT_TILES, S], BF16)
            v_r = p0.tile([128, SKT, HL, DH + 1], BF16)
            outT_r = p0.tile([128, QT_TILES, S], BF16)
            wo_r = p0.tile([128, QT_TILES, DM], BF16)
            ident_sb = p0.tile([128, 128], BF16)
            ones_col = p0.tile([128, 1], BF16)
            nc.vector.memset(ones_col[:], 1.0)

            with (
                tc.tile_pool(name="pa", bufs=1) as pa,
                tc.tile_pool(name="pa_t", bufs=4) as pa_t,
                tc.tile_pool(name="ps_b", bufs=1, space="PSUM") as ps_b,
                tc.tile_pool(name="pb_e", bufs=2) as pb_e,
                tc.tile_pool(name="pb_n", bufs=4) as pb_n,
                tc.tile_pool(name="pb_s", bufs=4) as pb_s,
                tc.tile_pool(name="pc_y", bufs=4) as pc_y,
            ):
                # PE pstate warm-up (cost model halves PE clock ~3us)
                wsrc = pa.tile([128, 512], BF16, tag="wsrc")
                nc.vector.memset(wsrc[:], 1.0)
                wps = ps_b.tile([128, 512], F32, tag="st", bufs=4,
                                name="wps")
                for _ in range(7):
                    nc.tensor.matmul(wps[:], wsrc[:, 0:128], wsrc[:],
                                     start=True, stop=True)

                # DMA order: wq then x chunks first so PE starts ASAP
                wq_r = pa.tile([128, KCH, DHL], BF16, tag="wqr")
                wq_re = wq.rearrange("(k p) n -> p k n", p=128)
                nc.sync.dma_start(wq_r[:, 0:2, :], wq_re[:, 0:2, :])
                xT_r = pa.tile([128, KCH, S], BF16)
                nc.sync.dma_start(xT_r[:, 0, 0:QB], xT[0:128, 0:QB])
                nc.sync.dma_start(xT_r[:, 0, QB:S], xT[0:128, QB:S])
                nc.sync.dma_start(xT_r[:, 1, 0:QB], xT[128:256, 0:QB])
                nc.sync.dma_start(xT_r[:, 1, QB:S], xT[128:256, QB:S])
                bq_sb = pa.tile([128, QT_TILES], F32, tag="bq")
                bk_sb = pa.tile([128, QT_TILES], F32, tag="bk")
                nc.sync.dma_start(bq_sb[:], bq[:, :])
                nc.sync.dma_start(bk_sb[:], bk[:, :])
                nc.sync.dma_start(wq_r[:, 2:3, :], wq_re[:, 2:3, :])
                nc.sync.dma_start(xT_r[:, 2, 0:QB], xT[256:384, 0:QB])
                nc.sync.dma_start(xT_r[:, 2, QB:S], xT[256:384, QB:S])
                nc.sync.dma_start(wq_r[:, 3:KCH, :], wq_re[:, 3:KCH, :])
                nc.sync.dma_start(xT_r[:, 3, 0:QB], xT[384:512, 0:QB])
                nc.sync.dma_start(xT_r[:, 3, QB:S], xT[384:512, QB:S])
                for k in range(4, KCH):
                    nc.sync.dma_start(xT_r[:, k, :], xT[k * 128:(k + 1) * 128, :])
                wk_r = pa.tile([128, KCH, DHL], BF16, tag="wkr")
                nc.sync.dma_start(
                    wk_r[:], wk.rearrange("(k p) n -> p k n", p=128))
                cos_sb = pa.tile([128, S], BF16)
                sin_sb = pa.tile([128, S], BF16)
                nc.sync.dma_start(cos_sb[:], cosT[:, :])
                nc.sync.dma_start(sin_sb[:], sinT[:, :])
                nc.sync.dma_start(ident_sb[:], ident[:, :])
                wv_r = pa.tile([128, KCH, DHL], BF16, tag="wvr")
                nc.sync.dma_start(
                    wv_r[:], wv.rearrange("(k p) n -> p k n", p=128))
                nc.sync.dma_start(wo_r[:], wo[:, :, :])

                # preload the exp ACT table while ACT is idle
                warm = pa.tile([1, 128], F32, tag="warm")
                nc.vector.memset(warm[:], 0.0)
                warm2 = pa.tile([1, 128], F32, tag="warm2")
                nc.scalar.activation(warm2[:], warm[:], EXP, scale=ACT_SCALE)

                # ================= PHASE A =================
                def st_tile(name):
                    return ps_b.tile([128, 512], F32, tag="st", bufs=4,
                                     name=name)

                def pv_tile(name):
                    return ps_b.tile([128, 2, 512], F32, tag="pv", bufs=2,
                                     name=name)

                def rope_evac(pair, mt, qb, b_sb, dest):
                    """PSUM halves 2x[128, 512] -> dest rope slice (fp16).

                    rotate-half is 4 partition-offset SBUF->SBUF DMA copies;
                    the rot signs live in the (pre-negated) sin table rows."""
                    q0 = qb * QB
                    qb_r = pa_t.tile([128, QB], BF16, tag="qbr")
                    for nq in range(2):
                        nc.scalar.activation(qb_r[:, nq * 512:(nq + 1) * 512],
                                             pair[nq], IDENT,
                                             bias=b_sb[:, mt:mt + 1])
                    qrot = pa_t.tile([128, QB], BF16, tag="qrot")
                    hh = DH // 2
                    for blk in range(4):
                        p0_ = blk * hh
                        src_ = p0_ + hh if blk % 2 == 0 else p0_ - hh
                        nc.sync.dma_start(qrot[p0_:p0_ + hh, :],
                                          qb_r[src_:src_ + hh, :])
                    t1 = pa_t.tile([128, QB], BF16, tag="t1")
                    nc.vector.tensor_mul(t1[:], qb_r[:],
                                         cos_sb[:, q0:q0 + QB])
                    t2 = pa_t.tile([128, QB], BF16, tag="t2")
                    nc.vector.tensor_mul(t2[:], qrot[:],
                                         sin_sb[:, q0:q0 + QB])
                    nc.vector.tensor_add(dest[:, mt, q0:q0 + QB],
                                         t1[:], t2[:])

                # Q projection: wave1 = qb0 groups k-outer (consume the x
                # stream), wave2 = qb1 groups dense (x fully resident)
                wave1 = [(0, 0), (1, 0)]
                wave2 = [(0, 1), (1, 1)]
                qps = {}
                for mt, qb in wave1:
                    for nq in range(2):
                        qps[(mt, qb, nq)] = st_tile(f"qp{mt}{qb}{nq}")
                for k in range(KCH):
                    for mt, qb in wave1:
                        for nq in range(2):
                            q0 = qb * QB + nq * 512
                            nc.tensor.matmul(
                                qps[(mt, qb, nq)][:],
                                wq_r[:, k, mt * 128:(mt + 1) * 128],
                                xT_r[:, k, q0:q0 + 512],
                                start=(k == 0), stop=(k == KCH - 1))
                # wave2 groups: (0,1) in st slots, (1,1) in a pv slot
                pvq = pv_tile("qpv")
                for i, (mt, qb) in enumerate(wave2):
                    for nq in range(2):
                        qps[(mt, qb, nq)] = (
                            st_tile(f"qp{mt}{qb}{nq}") if i == 0
                            else pvq[:, nq, :])
                    for k in range(KCH):
                        for nq in range(2):
                            q0 = qb * QB + nq * 512
                            nc.tensor.matmul(
                                qps[(mt, qb, nq)][:] if i == 0
                                else pvq[:, nq, :],
                                wq_r[:, k, mt * 128:(mt + 1) * 128],
                                xT_r[:, k, q0:q0 + 512],
                                start=(k == 0), stop=(k == KCH - 1))
                    if i == 0:
                        rope_evac([qps[(0, 0, 0)][:], qps[(0, 0, 1)][:]],
                                  0, 0, bq_sb, qrope_r)
                        rope_evac([qps[(1, 0, 0)][:], qps[(1, 0, 1)][:]],
                                  1, 0, bq_sb, qrope_r)
                rope_evac([qps[(0, 1, 0)][:], qps[(0, 1, 1)][:]],
                          0, 1, bq_sb, qrope_r)
                rope_evac([pvq[:, 0, :], pvq[:, 1, :]], 1, 1, bq_sb, qrope_r)

                # K projection, rope-evacs software-pipelined one behind
                kgroups = [(0, 0), (0, 1), (1, 0), (1, 1)]
                kps = {}
                for i, (mt, qb) in enumerate(kgroups):
                    kps[(mt, qb, 0)] = st_tile(f"kp{mt}{qb}0")
                    kps[(mt, qb, 1)] = st_tile(f"kp{mt}{qb}1")
                    for k in range(KCH):
                        for nq in range(2):
                            q0 = qb * QB + nq * 512
                            nc.tensor.matmul(
                                kps[(mt, qb, nq)][:],
                                wk_r[:, k, mt * 128:(mt + 1) * 128],
                                xT_r[:, k, q0:q0 + 512],
                                start=(k == 0), stop=(k == KCH - 1))
                    if i >= 1:
                        pmt, pqb = kgroups[i - 1]
                        rope_evac([kps[(pmt, pqb, 0)][:],
                                   kps[(pmt, pqb, 1)][:]],
                                  pmt, pqb, bk_sb, krope_r)
                g = kgroups[-1]
                rope_evac([kps[(g[0], g[1], 0)][:], kps[(g[0], g[1], 1)][:]],
                          g[0], g[1], bk_sb, krope_r)

                # V projection (natural layout)
                nc.vector.tensor_copy(
                    v_r[:, :, :, DH:DH + 1],
                    ones_col[:, None, None, :].broadcast_to([128, SKT, HL, 1]))
                for sk in range(SKT):
                    vps = ps_b.tile([128, DHL], F32, tag="st", bufs=4,
                                    name="vps")
                    for k in range(KCH):
                        nc.tensor.matmul(
                            vps[:], xT_r[:, k, sk * 128:(sk + 1) * 128],
                            wv_r[:, k, :],
                            start=(k == 0), stop=(k == KCH - 1))
                    nc.scalar.activation(
                        v_r[:, sk, :, 0:DH],
                        vps[:].rearrange("p (h d) -> p h d", h=HL), COPY)

                # ================= PHASE B + C =================
                # units: (chunk, head); per unit 16 score tiles -> exp ->
                # flipped-PV in two passes (qt 0,1 then 2,3).  Deferred work
                # from the previous unit is stitched into this unit's score
                # stream at fixed sk points chosen so no engine queue blocks:
                #   sk2:  prev unit's pvB matmuls (PE, deps long ready)
                #   sk6:  prev unit's pvB norm (DVE; pvB has stopped by now)
                #   sk8+: transposes (PE; norms done), one per sk
                #   sk11/13/15: one deferred phase-C y-tile each
                units = [(ch, h) for ch in range(NCHUNK) for h in range(HL)]
                stagings = {}   # (ch, mt) -> staging tile
                pend = {}       # ui -> deferred closures
                cq = []         # deferred phase-C y-tiles

                def make_unit(ui):
                    ch, h = units[ui]
                    q0 = ch * W
                    mt, half = h // 2, (h % 2) * DH
                    if (ch, mt) not in stagings:
                        stagings[(ch, mt)] = pb_s.tile(
                            [128, 4, 2, DH], BF16, tag="stg",
                            name=f"stg{ch}{mt}")
                    e_t = pb_e.tile([128, SKT, W], BF16, tag="ert",
                                    name=f"e{ui}")

                    def pv_mm(pv_t, sk, qt0):
                        for j in range(2):
                            qt = qt0 + j
                            nc.tensor.matmul(
                                pv_t[:, j, 0:DH + 1],
                                e_t[:, sk, qt * 128:(qt + 1) * 128],
                                v_r[:, sk, h, :],
                                start=(sk == 0), stop=(sk == SKT - 1))

                    def norm(pv_t, qt0):
                        stg = stagings[(ch, mt)]
                        den = pb_n.tile([128, 2], F32, tag="den")
                        nc.vector.tensor_copy(den[:], pv_t[:, :, DH:DH + 1])
                        rec = pb_n.tile([128, 2], F32, tag="rec")
                        nc.vector.reciprocal(rec[:], den[:])
                        nc.vector.tensor_mul(
                            stg[:, qt0:qt0 + 2, h % 2, :],
                            pv_t[:, :, 0:DH],
                            rec[:, :, None].broadcast_to([128, 2, DH]))

                    def transp(qt):
                        stg = stagings[(ch, mt)]
                        tr = ps_b.tile([128, 128], BF16, tag="st",
                                       bufs=4, name="tr")
                        nc.tensor.transpose(tr[:], stg[:, qt, :, :],
                                            ident_sb[:])
                        nc.vector.tensor_copy(
                            outT_r[:, mt, q0 + qt * 128:q0 + (qt + 1) * 128],
                            tr[:])

                    return ch, h, mt, half, q0, e_t, pv_mm, norm, transp

                def emit_phase_c(ch):
                    """y tiles for one 512-q chunk, deferred via cq."""
                    for qt in range(ch * 4, ch * 4 + 4):
                        def do_qt(qt=qt):
                            y_sb = pc_y.tile([128, DM], BF16, tag="ysb")
                            for c2 in range(2):
                                y_ps = st_tile("yps")
                                for kc in range(QT_TILES):
                                    nc.tensor.matmul(
                                        y_ps[:],
                                        outT_r[:, kc, qt * 128:(qt + 1) * 128],
                                        wo_r[:, kc, c2 * 512:(c2 + 1) * 512],
                                        start=(kc == 0),
                                        stop=(kc == QT_TILES - 1))
                                half_ap = y_sb[:, c2 * 512:(c2 + 1) * 512]
                                if c2 == 0:
                                    nc.scalar.activation(half_ap, y_ps[:],
                                                         COPY)
                                else:
                                    nc.vector.tensor_copy(half_ap, y_ps[:])
                            nc.sync.dma_start(
                                y[qt * 128:(qt + 1) * 128, :], y_sb[:])
                        cq.append(do_qt)

                for ui in range(len(units)):
                    ch, h, mt, half, q0, e_t, pv_mm, norm, transp = \
                        make_unit(ui)
                    kt_h = krope_r[half:half + DH, mt, :]
                    qt_h = qrope_r[half:half + DH, mt, :]
                    prev = pend.pop(ui - 1, None)
                    pvA = [None]
                    for sk in range(SKT):
                        st = ps_b.tile([128, W], F32, tag="st", bufs=4,
                                       name="st")
                        nc.tensor.matmul(
                            st[:], kt_h[:, sk * 128:(sk + 1) * 128],
                            qt_h[:, q0:q0 + W], start=True, stop=True)
                        eng = EXP_PAT[sk]
                        if eng == "a":
                            nc.scalar.activation(e_t[:, sk, :], st[:], EXP,
                                                 scale=ACT_SCALE)
                        else:
                            nc.vector.tensor_scalar(
                                out=e_t[:, sk, :].bitcast(I16), in0=st[:],
                                scalar1=SCHR_MUL, scalar2=SCHR_B,
                                op0=MULT, op1=ADD)
                        # stitch deferred prev-unit work into the stream
                        if prev is not None:
                            p_pv_mm, p_norm, p_transp, p_odd, p_last = prev
                            if sk == 2:
                                pvB = pv_tile(f"pvB{ui - 1}")
                                for psk in range(SKT):
                                    p_pv_mm(pvB, psk, 2)
                            elif sk == 6:
                                p_norm(pvB, 2)
                            elif p_odd and sk in (8, 9, 10):
                                p_transp(sk - 8)
                            elif p_odd and sk == 12:
                                p_transp(3)
                                if p_last:
                                    emit_phase_c(units[ui - 1][0])
                        if sk == 11 and cq:
                            cq.pop(0)()
                        if sk == 13 and cq:
                            cq.pop(0)()
                        if sk == 15 and cq:
                            cq.pop(0)()
                        if sk >= 4:
                            if pvA[0] is None:
                                pvA[0] = pv_tile(f"pvA{ui}")
                            pv_mm(pvA[0], sk - 4, 0)
                    for sk in range(SKT - 4, SKT):
                        pv_mm(pvA[0], sk, 0)
                    norm(pvA[0], 0)
                    pend[ui] = (pv_mm, norm, transp, h % 2 == 1, h == HL - 1)

                # tail: finish the last unit directly
                p_pv_mm, p_norm, p_transp, p_odd, p_last = \
                    pend.pop(len(units) - 1)
                pvB = pv_tile("pvB_last")
                for psk in range(SKT):
                    p_pv_mm(pvB, psk, 2)
                p_norm(pvB, 2)
                for qt in range(4):
                    p_transp(qt)
                emit_phase_c(NCHUNK - 1)
                while cq:
                    cq.pop(0)()
                if debug:
                    nc.sync.dma_start(qrope_d[:, :, :], qrope_r[:])
                    nc.sync.dma_start(krope_d[:, :, :], krope_r[:])
                    nc.sync.dma_start(v_d[:, :, :, :], v_r[:])
                    nc.sync.dma_start(outT_d[:, :, :], outT_r[:])

    nc.finalize()
    return nc


def _rope_tables():
    """cos/sin tables [128, S]; sin rows d with d%64 < 32 are NEGATED so the
    rotate-half signs ride in the table (the device rot is 4 unsigned
    partition-block DMA copies)."""
    inv_freq = 1.0 / (10000.0 ** (np.arange(0, DH, 2, dtype=np.float32) / DH))
    ang = np.arange(S, dtype=np.float32)[:, None] * inv_freq[None, :]
    sin = np.concatenate([np.sin(ang), np.sin(ang)], axis=-1)  # [S, DH]
    cos = np.concatenate([np.cos(ang), np.cos(ang)], axis=-1)
    sinT = np.ascontiguousarray(np.vstack([sin.T, sin.T]), dtype=np.float32)
    cosT = np.ascontiguousarray(np.vstack([cos.T, cos.T]), dtype=np.float32)
    signs = np.where((np.arange(128) % DH) < (DH // 2), -1.0, 1.0)
    sinT = sinT * signs[:, None].astype(np.float32)
    return sinT, cosT  # [128, S]


def _make_runner(nc):
    """Build a cached jitted SPMD executor (mirrors the multi-core tail of
    concourse.bass2jax.run_bass_via_pjrt so repeat calls skip recompiles)."""
    import jax
    import numpy as _np
    from jax.sharding import Mesh, PartitionSpec
    from jax.experimental.shard_map import shard_map
    from concourse import bass2jax, mybir as _mybir

    bass2jax.install_neuronx_cc_hook()

    partition_name = (
        nc.partition_id_tensor.name if nc.partition_id_tensor else None)
    in_names, out_names, out_avals, zero_shapes = [], [], [], []
    for alloc in nc.m.functions[0].allocations:
        if not isinstance(alloc, _mybir.MemoryLocationSet):
            continue
        name = alloc.memorylocations[0].name
        if alloc.kind == "ExternalInput":
            if name != partition_name:
                in_names.append(name)
        elif alloc.kind == "ExternalOutput":
            out_names.append(name)
            shape = tuple(alloc.tensor_shape)
            dtype = _mybir.dt.np(alloc.dtype)
            out_avals.append(jax.core.ShapedArray(shape, dtype))
            zero_shapes.append((shape, dtype))
    n_params = len(in_names)
    all_names = in_names + out_names
    if partition_name is not None:
        all_names = all_names + [partition_name]

    def _body(*args):
        operands = list(args)
        if partition_name is not None:
            operands.append(bass2jax.partition_id_tensor())
        outs = bass2jax._bass_exec_p.bind(
            *operands,
            out_avals=tuple(out_avals),
            in_names=tuple(all_names),
            out_names=tuple(out_names),
            lowering_input_output_aliases=(),
            sim_require_finite=True,
            sim_require_nnan=True,
            nc=nc,
        )
        return tuple(outs)

    devices = jax.devices()[:NCORES]
    mesh = Mesh(_np.asarray(devices), ("core",))
    n_outs = len(out_names)
    sharded = jax.jit(
        shard_map(
            _body, mesh=mesh,
            in_specs=(PartitionSpec("core"),) * (n_params + n_outs),
            out_specs=(PartitionSpec("core"),) * n_outs,
            check_rep=False,
        ),
        donate_argnums=tuple(range(n_params, n_params + n_outs)),
        keep_unused=True,
    )

    def run(in_maps):
        concat_in = [
            _np.concatenate([_np.asarray(m[name]) for m in in_maps], axis=0)
            for name in in_names
        ]
        concat_zeros = [
            _np.zeros((NCORES * s[0], *s[1:]), dt) for (s, dt) in zero_shapes
        ]
        out_arrs = sharded(*concat_in, *concat_zeros)
        return [
            {
                name: _np.asarray(out_arrs[i]).reshape(
                    NCORES, *out_avals[i].shape)[c]
                for i, name in enumerate(out_names)
            }
            for c in range(NCORES)
        ]

    return run


def _get_runner():
    if "runner" not in _CACHE:
        nc = _build_nc()
        _CACHE["nc"] = nc
        _CACHE["runner"] = _make_runner(nc)
    return _CACHE["runner"]


def _bf16(a):
    return np.asarray(a, dtype=np.float32).astype(np.float16)


def make_in_maps(x, wq, bq, wk, bk, wv, bv, wo, bo):
    """Build the 8 per-core input dicts from full inputs."""
    x = np.asarray(x, dtype=np.float32)
    if "tables" not in _CACHE:
        _CACHE["tables"] = _rope_tables()
    sinT, cosT = _CACHE["tables"]
    eye = np.eye(128, dtype=np.float16)
    in_maps = []
    for c in range(NCORES):
        b, hg = divmod(c, HL)
        sl = slice(hg * DHL, (hg + 1) * DHL)
        in_maps.append({
            "xT": _bf16(x[b].T),
            "wq": _bf16(np.asarray(wq, np.float32)[:, sl] * QSCALE),
            "wk": _bf16(np.asarray(wk, np.float32)[:, sl]),
            "wv": _bf16(np.asarray(wv, np.float32)[:, sl]),
            "wo": _bf16(
                np.asarray(wo, np.float32)[sl, :].reshape(QT_TILES, 128, DM)
                .transpose(1, 0, 2)),
            "bq": np.ascontiguousarray(
                (np.asarray(bq, np.float32)[sl] * QSCALE)
                .reshape(QT_TILES, 128).T),
            "bk": np.ascontiguousarray(
                np.asarray(bk, np.float32)[sl].reshape(QT_TILES, 128).T),
            "cosT": _bf16(cosT),
            "sinT": _bf16(sinT),
            "ident": eye,
        })
    return in_maps


def kernel(x, wq, bq, wk, bk, wv, bv, wo, bo):
    runner = _get_runner()
    in_maps = make_in_maps(x, wq, bq, wk, bk, wv, bv, wo, bo)
    results = runner(in_maps)
    bo = np.asarray(bo, dtype=np.float32)
    # the device computes attention with V's bias omitted; its contribution
    # to y is the constant row bv @ wo, folded into the host-side bias add
    const_row = (np.asarray(bv, np.float32) @ np.asarray(wo, np.float32)
                 + bo).astype(np.float32)
    out = np.empty((B, S, DM), dtype=np.float32)
    for b in range(B):
        acc = results[b * HL + 0]["y"].astype(np.float32, copy=True)
        for hg in range(1, HL):
            acc += results[b * HL + hg]["y"]
        out[b] = acc + const_row[None, :]
    return out
